# revision 40
# baseline (speedup 1.0000x reference)
"""Trainium2 Bass kernel for LlamaAttention (B=2, S=2048, D=2048, H=16, HD=128).

Sharding: batch-split x head tensor-parallel. Cores 0-3 take batch 0, cores
4-7 batch 1; within a group each core owns 4 heads (512 feature columns of
Wq/Wk/Wv, 512 rows of Wo). Each core computes q/k/v projections + rope for
its heads, causal-masked softmax attention, AV, and a partial output
projection; the host sums the 4 partials per batch.

All matmul operands are fp16 (PSUM accumulation stays fp32; the softmax
denominator pipeline is fp32). Device layout is feature-major: hs^T [D, S]
so contraction dims land on SBUF partitions; attention runs on
scores^T = k-block^T @ q^T tiles, two heads per flight, with scores/exp
restricted to the causally-live column range per key tile. Softmax uses
the host-shifted mask trick (P = exp(scale*S) * em, em in [0,1], unique
boundary tiles resident in SBUF); row sums land on partitions 0/32/64/96
of one PSUM bank via ones-column stationaries; normalization is
scalar-copy (PSUM offsets mis-read by the custom DVE op) ->
reciprocal_approx_fast -> gpsimd broadcast folded into the eviction.
V never spills to DRAM: PE-transposed, SBUF-resident. hs/weights stream
in kt-quarter tiles (deps are tile-granular) with the first quarter
halved so the PE starts ~12us in. Output projection for token block i is
emitted between attention flights of block i+1; evictions alternate
vector/scalar and output DMAs alternate the two HWDGE queues.
"""

import os
import sys
from contextlib import ExitStack

import numpy as np

for _p in ("/opt/trn_rl_repo",):
    if _p not in sys.path:
        sys.path.insert(0, _p)

import ml_dtypes  # noqa: E402

import concourse.bass as bass  # noqa: E402,F401
import concourse.tile as tile  # noqa: E402
from concourse import bacc, mybir  # noqa: E402
from concourse.masks import make_identity  # noqa: E402

B, S, D, H, HD = 2, 2048, 2048, 16, 128
NCORES = 8
CPG = 4                      # cores per batch group
HPC = H // CPG               # 4 heads per core
JC = HPC * HD                # 512 per-core feature columns
P = 128
TB = 512                     # token block for projections
NTB = S // TB                # 4 per core (one batch)
KT = D // P                  # 16 contraction tiles
TQB = 512                    # tq block in attention
NTQB = S // TQB              # 4
NTK = S // P                 # 16 tk tiles
NBW = 256                    # output-projection free-dim block
SCALE = 1.0 / float(np.sqrt(HD))
ROPE_THETA = 10000.0

F32 = mybir.dt.float32
F16 = mybir.dt.float16
F8 = mybir.dt.float8e4

# tile classes
CLS_SKIP, CLS_ZERO, CLS_MIXED = 0, 1, 2

_prog_cache: dict[tuple, object] = {}


def _build_program(cls: np.ndarray, uidseq: tuple, n_uniq: int):
    """cls: [NTK, NTQB] int8 tile classes (shared by both batches).
    uidseq: for each mixed tile in (tqb, tk) scan order, the index of its
    mask pattern inside the resident em tensor."""
    nc = bacc.Bacc(
        "TRN2",
        target_bir_lowering=False,
        debug=False,
        enable_asserts=True,
        num_devices=NCORES,
    )

    hsT_d = nc.dram_tensor("hsT", [D, S], F16, kind="ExternalInput").ap()
    wq_d = nc.dram_tensor("wq", [P, KT * JC], F16, kind="ExternalInput").ap()
    wk_d = nc.dram_tensor("wk", [P, KT * JC], F16, kind="ExternalInput").ap()
    wv_d = nc.dram_tensor("wv", [P, KT * JC], F16, kind="ExternalInput").ap()
    wo_d = nc.dram_tensor("wo", [P, HPC * D], F16, kind="ExternalInput").ap()
    cos_d = nc.dram_tensor("cosT", [HD, S], F16, kind="ExternalInput").ap()
    sin_d = nc.dram_tensor("sinT", [HD, S], F16, kind="ExternalInput").ap()
    em_d = nc.dram_tensor("emU", [P, max(n_uniq, 1) * TQB], F16,
                          kind="ExternalInput").ap()
    out_d = nc.dram_tensor("out", [S, D], F16, kind="ExternalOutput").ap()

    hsT_v = hsT_d.rearrange("(kt p) t -> p kt t", p=P)       # [128, 16, 2048]

    # mixed-tile (uid, first-live-col) lookup in (tqb, tk) scan order
    uid_of = {}
    lo_of = {}
    ui = 0
    for tqb in range(NTQB):
        for tk in range(NTK):
            if cls[tk, tqb] == CLS_MIXED:
                uid_of[(tk, tqb)], lo_of[(tk, tqb)] = uidseq[ui]
                ui += 1
    assert ui == len(uidseq)

    FH = 2                       # heads per attention flight
    NFL = HPC // FH              # 2 flights

    with tile.TileContext(nc) as tc, ExitStack() as ctx:
        # ---------- long-lived tiles ----------
        persist = ctx.enter_context(tc.tile_pool(name="persist", bufs=1))
        # per-token-block tiles: deps are tile-granular, so block-sharding
        # lets attention for block t start without waiting on later blocks
        qTb = [persist.tile([P, HPC * TB], F16, name=f"qTb{t}")
               for t in range(NTB)]
        kTb = [persist.tile([P, HPC * TB], F16, name=f"kTb{t}")
               for t in range(NTB)]
        vTb = [persist.tile([P, HPC * 4 * P], F16, name=f"vTb{t}")
               for t in range(NTB)]
        ident = persist.tile([P, P], F16)
        # ones2[:, hh, :]: column 32*hh all-ones. Lands flight-head hh's
        # exp-sum on partition 32*hh of a shared [33, TQB] PSUM region
        # (engines may only address partition offsets 0/32/64/96).
        ones4 = persist.tile([P, HPC * 97], F16)
        ones4_v = ones4[:].rearrange("p (h c) -> p h c", h=HPC)


        qTb_v = [t[:].rearrange("p (h t) -> p h t", h=HPC) for t in qTb]
        kTb_v = [t[:].rearrange("p (h t) -> p h t", h=HPC) for t in kTb]
        vTb_v = [t[:].rearrange("p (h m j) -> p h m j", h=HPC, m=4)
                 for t in vTb]

        aTb = [persist.tile([P, HPC * TQB], F16, name=f"aTb{t}")
               for t in range(NTQB)]              # attn out^T per block
        aTb_v = [t[:].rearrange("p (h t) -> p h t", h=HPC) for t in aTb]
        wo_s = persist.tile([P, HPC * D], F16)
        wo_sv = wo_s[:].rearrange("p (h n) -> p h n", h=HPC)
        em_s = persist.tile([P, max(n_uniq, 1) * TQB], F16)

        dbg_rbcs: list = []
        dbg_extra: list = []

        # ---------- fused phase: projections + attention per token block ----
        KQ = 4                  # kt tiles per quarter; deps are tile-granular
        NQ = KT // KQ
        sps = ctx.enter_context(
            tc.tile_pool(name="spsum", bufs=3, space="PSUM"))
        phase1 = ExitStack()
        wpool = phase1.enter_context(tc.tile_pool(name="wpool", bufs=1))
        cspool = phase1.enter_context(tc.tile_pool(name="cspool", bufs=1))
        hstp = phase1.enter_context(tc.tile_pool(name="hstp", bufs=2 * NQ))
        stg = phase1.enter_context(tc.tile_pool(name="stg", bufs=2))
        vstgp = phase1.enter_context(tc.tile_pool(name="vstg", bufs=HPC))
        pps = phase1.enter_context(
            tc.tile_pool(name="ppsum", bufs=3, space="PSUM"))
        vtp = phase1.enter_context(
            tc.tile_pool(name="vtpsum", bufs=2, space="PSUM"))
        if True:
            # weights and hs stream in kt-quarters so the PE can start after
            # ~1MB instead of waiting for whole-tile DMAs
            w_tiles = {w: [wpool.tile([P, KQ * JC], F16, name=f"w_{w}{qq}")
                           for qq in range(NQ)]
                       for w in ("v", "q", "k")}
            w_dram = {"v": wv_d, "q": wq_d, "k": wk_d}
            cos_s = cspool.tile([HD, S], F16)
            sin_s = cspool.tile([HD, S], F16)

            def load_hst_quarter(tb, qq):
                hq = hstp.tile([P, KQ * TB], F16, tag="hst")
                nc.sync.dma_start(
                    hq[:].rearrange("p (kt t) -> p kt t", t=TB),
                    hsT_v[:, qq * KQ:(qq + 1) * KQ,
                          tb * TB:(tb + 1) * TB],
                )
                return hq[:].rearrange("p (kt t) -> p kt t", t=TB)

            def load_w_quarter(w, qq):
                nc.sync.dma_start(
                    w_tiles[w][qq][:],
                    w_dram[w][:, qq * KQ * JC:(qq + 1) * KQ * JC])

            # interleaved, and the very first quarter split in halves on
            # separate DMA queues, so the first matmul is gated on ~0.5MB
            hst_q = []
            for qq in range(NQ):
                if qq == 0:
                    hq = hstp.tile([P, KQ * TB], F16, tag="hst")
                    hqv = hq[:].rearrange("p (kt t) -> p kt t", t=TB)
                    for hf in range(2):
                        nc.sync.dma_start(
                            hqv[:, 2 * hf:2 * hf + 2, :],
                            hsT_v[:, 2 * hf:2 * hf + 2, 0:TB])
                    hst_q.append(hqv)
                    for hf in range(2):
                        nc.sync.dma_start(
                            w_tiles["v"][0][:, hf * 2 * JC:(hf + 1) * 2 * JC],
                            w_dram["v"][:, hf * 2 * JC:(hf + 1) * 2 * JC])
                    continue
                hst_q.append(load_hst_quarter(0, qq))
                load_w_quarter("v", qq)
            nc.sync.dma_start(cos_s[:], cos_d)
            nc.sync.dma_start(sin_s[:], sin_d)
            for w in ("q", "k"):
                for qq in range(NQ):
                    load_w_quarter(w, qq)
            nc.scalar.dma_start(em_s[:], em_d)
            nc.scalar.dma_start(wo_s[:], wo_d)

            w_views = {
                w: [t[:].rearrange("p (kt j) -> p kt j", j=JC)
                    for t in w_tiles[w]]
                for w in ("v", "q", "k")
            }

            def proj_group(wname, j2, hst_v, dst_ps):
                for kt in range(KT):
                    nc.tensor.matmul(
                        dst_ps[:],
                        lhsT=w_views[wname][kt // KQ][
                            :, kt % KQ, j2 * P:(j2 + 1) * P],
                        rhs=hst_v[kt // KQ][:, kt % KQ, :],
                        start=(kt == 0),
                        stop=(kt == KT - 1),
                    )

            def rope_evict(j2, dst, ps, tsl):
                # out[:64] = x1*cos - x2*sin ; out[64:] = x2*cos + x1*sin
                c1 = stg.tile([P, TB], F32, tag="ropeA")
                c2 = stg.tile([P, TB], F32, tag="ropeB")
                nc.vector.tensor_mul(c1[:], ps[:], cos_s[:, tsl])
                nc.vector.tensor_mul(
                    c2[0:64, :], ps[64:128, :], sin_s[0:64, tsl])
                nc.vector.tensor_mul(
                    c2[64:128, :], ps[0:64, :], sin_s[64:128, tsl])
                nc.vector.tensor_sub(dst[0:64, :], c1[0:64, :], c2[0:64, :])
                nc.vector.tensor_add(
                    dst[64:128, :], c1[64:128, :], c2[64:128, :])

            def attention_block(tqb, sps, sups, ptp, ptmp, smp, rbcp,
                                emit_oproj_groups):
                tq0 = tqb * TQB
                live = [tk for tk in range(NTK) if cls[tk, tqb] != CLS_SKIP]
                L = len(live)
                los = [lo_of.get((tk, tqb), 0) for tk in live]

                # pair consecutive full (ZERO) tiles for fp8 DoubleRow
                # denominator sums; mask tiles keep the fp16 ones-matmul
                pair_second = {}
                zrun = []  # fp8-DR denominator rejected: softmax
                # concentration lets a dominant key's e4m3 error (~6%)
                # reach the output unaveraged
                zi = 0
                while zi + 1 < len(zrun):
                    if zrun[zi + 1] == zrun[zi] + 1:
                        pair_second[zrun[zi + 1]] = zrun[zi]
                        zi += 2
                    else:
                        zi += 1
                sums_events = [i for i in range(L)
                               if i not in pair_second.values()]

                def av_sums(i, hh, h, pt, sums, fl):
                    # one accumulation window per PSUM bank: tile 0 zeroes
                    # the full range (los[0] == 0 guaranteed), the last tile
                    # closes it; middle tiles write their live subrange only
                    first = (i == 0)
                    last = (i == L - 1)
                    lo = 0 if first else los[i]
                    nc.tensor.matmul(
                        o_ps[hh][:, lo:],
                        lhsT=vTb_v[live[i] // 4][:, h, live[i] % 4, :],
                        rhs=pt[:, lo:],
                        start=first, stop=last,
                    )
                    if i in pair_second.values():
                        return
                    s_first = (i == sums_events[0])
                    s_last = (i == sums_events[-1])
                    nc.tensor.matmul(
                        sums[:, lo:],
                        lhsT=ones4_v[:, fl * FH + hh, :],
                        rhs=pt[:, lo:],
                        start=(s_first and first and hh == 0),
                        stop=(s_last and last and hh == FH - 1),
                    )

                for fl in range(NFL):
                    heads = range(fl * FH, (fl + 1) * FH)
                    sums = sups.tile([97, TQB], F32, tag="sums")
                    pts = [[None] * FH for _ in range(2)]
                    for i, tk in enumerate(live):
                        lo = los[i]
                        for hh, h in enumerate(heads):
                            st = sps.tile([P, TQB], F32, tag="st")
                            nc.tensor.matmul(
                                st[:, lo:],
                                lhsT=kTb_v[tk // 4][
                                    :, h, (tk % 4) * P:(tk % 4 + 1) * P],
                                rhs=qTb_v[tqb][:, h, lo:],
                                start=True, stop=True,
                            )
                            pt = ptp.tile([P, TQB], F16, tag="pt")
                            nc.scalar.activation(
                                pt[:, lo:], st[:, lo:],
                                mybir.ActivationFunctionType.Exp,
                                scale=SCALE,
                            )
                            if cls[tk, tqb] == CLS_MIXED:
                                # out-of-place: PE never sees pre-mask pt
                                u = uid_of[(tk, tqb)]
                                ptm = ptmp.tile([P, TQB], F16, tag="ptm")
                                nc.vector.tensor_mul(
                                    ptm[:, lo:], pt[:, lo:],
                                    em_s[:, u * TQB + lo:(u + 1) * TQB])
                                pt = ptm
                            pts[i % 2][hh] = pt
                            if i > 0:
                                av_sums(i - 1, hh, h,
                                        pts[(i - 1) % 2][hh], sums, fl)
                    for hh, h in enumerate(heads):
                        av_sums(L - 1, hh, h, pts[(L - 1) % 2][hh], sums, fl)
                    dbg_sums = None
                    if os.environ.get("KERNEL_DEBUG") and tqb == NTQB - 1:
                        dbg_sums = persist.tile([97, TQB], F32)
                        nc.scalar.copy(dbg_sums[:], sums[:])
                        dbg_extra.append((f"dbg_sums{fl}", dbg_sums))
                    for hh, h in enumerate(heads):
                        # reciprocal_approx_fast mis-reads PSUM partition
                        # offsets != 0 on HW: stage the row to partition 0
                        sr = smp.tile([1, TQB], F32, tag="sr")
                        r0 = 32 * (fl * FH + hh)
                        nc.scalar.copy(sr[:], sums[r0:r0 + 1, :])
                        rc = smp.tile([1, TQB], F32, tag="rc")
                        nc.vector.reciprocal_approx_fast(rc[:], sr[:])
                        rbc = rbcp.tile([P, TQB], F32, tag="rbc")
                        nc.gpsimd.partition_broadcast(rbc[:], rc[:])
                        if dbg_sums is not None:
                            dbg_rbcs.append(rbc)
                        nc.vector.tensor_mul(
                            aTb_v[tqb][:, h, :], o_ps[hh][:], rbc[:])
                    emit_oproj_groups(8)

            for tb in range(NTB):
                if tb > 0:
                    hst_q = [load_hst_quarter(tb, qq) for qq in range(NQ)]
                tsl = slice(tb * TB, (tb + 1) * TB)
                # v first: its fp16 staging copies run while q projects; the
                # v chunks transpose on the DMA xbar (not the PE)
                vss = []
                for j2 in range(HPC):
                    ps = pps.tile([P, TB], F32, tag="pp")
                    proj_group("v", j2, hst_q, ps)
                    vs = vstgp.tile([P, TB], F16, tag="vstage")
                    nc.scalar.copy(vs[:], ps[:])
                    vss.append(vs)
                    if tb == 0 and j2 == 0:
                        # deferred off the startup critical path
                        make_identity(nc, ident[:])
                        nc.gpsimd.memset(ones4[:], 0.0)
                        for h4 in range(HPC):
                            nc.gpsimd.memset(
                                ones4_v[:, h4, 32 * h4:32 * h4 + 1], 1.0)
                for j2 in range(HPC):
                    ps = pps.tile([P, TB], F32, tag="pp")
                    proj_group("q", j2, hst_q, ps)
                    rope_evict(j2, qTb_v[tb][:, j2, :], ps, tsl)
                for j2 in range(HPC):
                    for k in range(TB // P):
                        vps = vtp.tile([P, P], F16, tag="vt")
                        nc.tensor.matmul(
                            vps[:],
                            lhsT=vss[j2][:, k * P:(k + 1) * P],
                            rhs=ident[:],
                            is_transpose=True,
                        )
                        nc.scalar.copy(vTb_v[tb][:, j2, k, :], vps[:])
                for j2 in range(HPC):
                    ps = pps.tile([P, TB], F32, tag="pp")
                    proj_group("k", j2, hst_q, ps)
                    rope_evict(j2, kTb_v[tb][:, j2, :], ps, tsl)
            phase1.close()

        # ---------- phase 2: attention with interleaved output projection ----
        with tc.tile_pool(name="ptp", bufs=8) as ptp, \
             tc.tile_pool(name="ptmp", bufs=4) as ptmp, \
             tc.tile_pool(name="smp", bufs=2) as smp, \
             tc.tile_pool(name="rbcp", bufs=4) as rbcp, \
             tc.tile_pool(name="ostg", bufs=3) as ostgp, \
             tc.tile_pool(name="opsum", bufs=2, space="PSUM") as ops, \
             tc.tile_pool(name="avpsum", bufs=1, space="PSUM") as avp, \
             tc.tile_pool(name="supsum", bufs=1, space="PSUM") as sups:
            o_ps = [avp.tile([P, TQB], F32, tag=f"av{hh}", name=f"o_ps{hh}")
                    for hh in range(FH)]

            pending: list = []
            ng = [0]

            def emit_oproj_groups(n):
                for _ in range(min(n, len(pending))):
                    tb32, nb = pending.pop(0)
                    pso = ops.tile([P, TQB], F32, tag="op")
                    for j2 in range(HPC):
                        nc.tensor.matmul(
                            pso[:],
                            lhsT=aTb_v[tb32 // 4][
                                :, j2, (tb32 % 4) * P:(tb32 % 4 + 1) * P],
                            rhs=wo_sv[:, j2, nb * TQB:(nb + 1) * TQB],
                            start=(j2 == 0), stop=(j2 == HPC - 1),
                        )
                    og = ostgp.tile([P, TQB], F16, tag="og")
                    # alternate eviction engine to balance load
                    if ng[0] % 2 == 0:
                        nc.vector.tensor_copy(og[:], pso[:])
                    else:
                        nc.scalar.copy(og[:], pso[:])
                    (nc.sync if ng[0] % 2 == 0 else nc.scalar).dma_start(
                        out_d[tb32 * P:(tb32 + 1) * P,
                              nb * TQB:(nb + 1) * TQB],
                        og[:],
                    )
                    ng[0] += 1

            for tqb in range(NTQB):
                attention_block(tqb, sps, sups, ptp, ptmp, smp, rbcp,
                                emit_oproj_groups)
                pending += [(tb32, nb)
                            for tb32 in range(tqb * (TQB // P),
                                              (tqb + 1) * (TQB // P))
                            for nb in range(D // TQB)]
            emit_oproj_groups(len(pending))

            if os.environ.get("KERNEL_DEBUG"):
                for nm, tl in (("dbg_qT", qTb), ("dbg_kT", kTb),
                               ("dbg_vT", vTb), ("dbg_aT", aTb)):
                    w = tl[0][:].shape[1]
                    dd = nc.dram_tensor(
                        nm, [P, len(tl) * w], F16, kind="ExternalOutput").ap()
                    for ti, t in enumerate(tl):
                        nc.sync.dma_start(dd[:, ti * w:(ti + 1) * w], t[:])
                for hh, rb in enumerate(dbg_rbcs):
                    dd = nc.dram_tensor(
                        f"dbg_rbc{hh}", [P, TQB], F32,
                        kind="ExternalOutput").ap()
                    nc.sync.dma_start(dd, rb[:])
                for nm, t in dbg_extra:
                    dd = nc.dram_tensor(
                        nm, list(t[:].shape), F32, kind="ExternalOutput").ap()
                    nc.sync.dma_start(dd, t[:])

    nc.compile()
    return nc


def _host_prep(hidden_states, attention_mask, position_ids):
    hs2 = np.asarray(hidden_states, dtype=np.float32).reshape(B * S, D)
    hsT = np.ascontiguousarray(hs2.T).astype(ml_dtypes.float16 if False
                                             else np.float16)  # [D, B*S]

    # rope tables gathered by position_ids, feature-major
    inv_freq = 1.0 / (ROPE_THETA ** (np.arange(0, HD, 2, dtype=np.float32) / HD))
    pos = np.asarray(position_ids).astype(np.int64)
    maxpos = int(pos.max()) + 1
    t_ar = np.arange(maxpos, dtype=np.float32)
    freqs = np.outer(t_ar, inv_freq)
    emb = np.concatenate([freqs, freqs], axis=-1)           # [maxpos, 128]
    cos_tab = np.cos(emb).astype(np.float32)
    sin_tab = np.sin(emb).astype(np.float32)
    cosT = [np.ascontiguousarray(cos_tab[pos[b]].T).astype(np.float16)
            for b in range(B)]                               # [HD, S] each
    sinT = [np.ascontiguousarray(sin_tab[pos[b]].T).astype(np.float16)
            for b in range(B)]

    # shifted-exp mask, transposed per batch, tile classification + dedup
    m = np.asarray(attention_mask, dtype=np.float32)[:, 0]  # [B, tq, tk]
    rowmax = m.max(axis=-1, keepdims=True)
    em = np.exp(m - rowmax)                                 # [B, tq, tk] in [0,1]
    emT = np.ascontiguousarray(em.transpose(0, 2, 1))       # [B, tk, tq]
    emr = emT.reshape(B, NTK, P, NTQB, TQB)
    tmax = emr.max(axis=(2, 4))                             # [B, NTK, NTQB]
    tmin = emr.min(axis=(2, 4))
    cls_b = np.full((B, NTK, NTQB), CLS_MIXED, dtype=np.int8)
    cls_b[tmax == 0.0] = CLS_SKIP
    cls_b[(tmin == 1.0) & (tmax == 1.0)] = CLS_ZERO
    # merge across batches: process if any batch needs it; mixed if classes
    # differ or any is mixed (em data is per-batch anyway)
    cls = np.maximum(cls_b[0], cls_b[1])
    cls[cls_b[0] != cls_b[1]] = CLS_MIXED
    # guard: a fully-skipped tq column would leave PSUM unwritten
    for tqb in range(NTQB):
        if (cls[:, tqb] == CLS_SKIP).all():
            cls[0, tqb] = CLS_MIXED

    # per-batch unique mixed-tile patterns + first-live-column, scan order
    uidseqs, uniqs, loseqs = [], [], []
    for b in range(B):
        seen = {}
        seq = []
        los = []
        tiles = []
        for tqb in range(NTQB):
            for tk in range(NTK):
                if cls[tk, tqb] != CLS_MIXED:
                    continue
                blk = np.ascontiguousarray(
                    emT[b, tk * P:(tk + 1) * P,
                        tqb * TQB:(tqb + 1) * TQB]).astype(np.float16)
                key = blk.tobytes()
                if key not in seen:
                    seen[key] = len(tiles)
                    tiles.append(blk)
                seq.append(seen[key])
                nzc = blk.astype(bool).any(axis=0)
                los.append(int(np.argmax(nzc)) if nzc.any() else 0)
        uidseqs.append(tuple(seq))
        loseqs.append(los)
        uniqs.append(tiles)
    if uidseqs[0] != uidseqs[1]:
        # fall back to no dedup: sequential uids shared by construction
        nm = len(uidseqs[0])
        seqs = tuple(range(nm))
        uidseqs = [seqs, seqs]
        uniqs = [
            [np.ascontiguousarray(
                emT[b, tk * P:(tk + 1) * P, tqb * TQB:(tqb + 1) * TQB]
             ).astype(np.float16)
             for tqb in range(NTQB) for tk in range(NTK)
             if cls[tk, tqb] == CLS_MIXED]
            for b in range(B)
        ]
    # live-column offsets must agree across batches, else no restriction
    los = [a if a == bb else 0 for a, bb in zip(loseqs[0], loseqs[1])]
    # per tq block: offsets must start at 0 (zero-class tiles are implicit 0)
    # and be non-decreasing in tk scan order, else disable for that block
    ui = 0
    for tqb in range(NTQB):
        idxs = []
        full = []
        for tk in range(NTK):
            if cls[tk, tqb] == CLS_MIXED:
                full.append(los[ui])
                idxs.append(ui)
                ui += 1
            elif cls[tk, tqb] == CLS_ZERO:
                full.append(0)
        ok = all(full[i] <= full[i + 1] for i in range(len(full) - 1))
        if full and full[0] != 0:
            ok = False
        if not ok:
            for j in idxs:
                los[j] = 0
    uidseq = tuple(zip(uidseqs[0], los))
    n_uniq = max(len(uniqs[0]), len(uniqs[1]), 1)
    em_u = []
    for b in range(B):
        buf = np.zeros((P, n_uniq * TQB), dtype=np.float16)
        for u, blk in enumerate(uniqs[b]):
            buf[:, u * TQB:(u + 1) * TQB] = blk
        em_u.append(buf)
    return hsT, cosT, sinT, cls, uidseq, n_uniq, em_u


def _sb_w(w):  # [D, JC] fp32 -> SBUF layout [128, KT*JC] fp16
    return np.ascontiguousarray(
        w.reshape(KT, P, JC).transpose(1, 0, 2).reshape(P, KT * JC)
    ).astype(np.float16)


def kernel(hidden_states, attention_mask, position_ids, Wq, Wk, Wv, Wo):
    hsT, cosT, sinT, cls, uidseq, n_uniq, em_u = _host_prep(
        hidden_states, attention_mask, position_ids)

    key = (cls.tobytes(), uidseq, n_uniq)
    if key not in _prog_cache:
        _prog_cache[key] = _build_program(cls, uidseq, n_uniq)
    nc = _prog_cache[key]

    Wq = np.asarray(Wq, dtype=np.float32)
    Wk = np.asarray(Wk, dtype=np.float32)
    Wv = np.asarray(Wv, dtype=np.float32)
    Wo = np.asarray(Wo, dtype=np.float32)

    in_maps = []
    for c in range(NCORES):
        b, g = c // CPG, c % CPG
        jsl = slice(g * JC, (g + 1) * JC)
        m = {
            "hsT": np.ascontiguousarray(hsT[:, b * S:(b + 1) * S]),
            "wq": _sb_w(Wq[:, jsl]),
            "wk": _sb_w(Wk[:, jsl]),
            "wv": _sb_w(Wv[:, jsl]),
            "wo": np.ascontiguousarray(
                Wo[jsl, :].reshape(HPC, P, D).transpose(1, 0, 2)
                .reshape(P, HPC * D)).astype(np.float16),
            "cosT": cosT[b],
            "sinT": sinT[b],
            "emU": em_u[b],
        }
        in_maps.append(m)

    if os.environ.get("KERNEL_SIM"):
        from concourse.bass_interp import CoreSim
        outs = []
        for c in range(int(os.environ.get("KERNEL_SIM_CORES", "1"))):
            sim = CoreSim(nc, require_finite=False, require_nnan=True)
            for k, v in in_maps[c].items():
                sim.tensor(k)[:] = v
            sim.simulate(check_with_hw=False)
            outs.append(np.array(sim.tensor("out")).astype(np.float32))
        kernel.last_sim_partials = outs
        total = np.zeros((B, S, D), dtype=np.float32)
        for c, o in enumerate(outs):
            total[c // CPG] += o
        return total

    from concourse.bass_utils import run_bass_kernel_spmd
    trace = bool(os.environ.get("KERNEL_TRACE"))
    res = run_bass_kernel_spmd(
        nc, in_maps, core_ids=list(range(NCORES)), trace=trace)
    if trace and res.exec_time_ns is not None:
        print(f"HW exec time: {res.exec_time_ns} ns")
        kernel.last_exec_time_ns = res.exec_time_ns
        kernel.last_trace = res.instructions_and_trace
    total = np.zeros((B, S, D), dtype=np.float32)
    for c, r in enumerate(res.results):
        total[c // CPG] += np.asarray(r["out"], dtype=np.float32)
    return total


# revision 41
# speedup vs baseline: 1.0088x; 1.0088x over previous
"""Trainium2 Bass kernel for LlamaAttention (B=2, S=2048, D=2048, H=16, HD=128).

Sharding: batch-split x head tensor-parallel. Cores 0-3 take batch 0, cores
4-7 batch 1; within a group each core owns 4 heads (512 feature columns of
Wq/Wk/Wv, 512 rows of Wo). Each core computes q/k/v projections + rope for
its heads, causal-masked softmax attention, AV, and a partial output
projection; the host sums the 4 partials per batch.

All matmul operands are fp16 (PSUM accumulation stays fp32; the softmax
denominator pipeline is fp32). Device layout is feature-major: hs^T [D, S]
so contraction dims land on SBUF partitions; attention runs on
scores^T = k-block^T @ q^T tiles, two heads per flight, with scores/exp
restricted to the causally-live column range per key tile. Softmax uses
the host-shifted mask trick (P = exp(scale*S) * em, em in [0,1], unique
boundary tiles resident in SBUF); row sums land on partitions 0/32/64/96
of one PSUM bank via ones-column stationaries; normalization is
scalar-copy (PSUM offsets mis-read by the custom DVE op) ->
reciprocal_approx_fast -> gpsimd broadcast folded into the eviction.
V never spills to DRAM: PE-transposed, SBUF-resident. hs/weights stream
in kt-quarter tiles (deps are tile-granular) with the first quarter
halved so the PE starts ~12us in. Output projection for token block i is
emitted between attention flights of block i+1; evictions alternate
vector/scalar and output DMAs alternate the two HWDGE queues.
"""

import os
import sys
from contextlib import ExitStack

import numpy as np

for _p in ("/opt/trn_rl_repo",):
    if _p not in sys.path:
        sys.path.insert(0, _p)

import ml_dtypes  # noqa: E402

import concourse.bass as bass  # noqa: E402,F401
import concourse.tile as tile  # noqa: E402
from concourse import bacc, mybir  # noqa: E402
from concourse.masks import make_identity  # noqa: E402

B, S, D, H, HD = 2, 2048, 2048, 16, 128
NCORES = 8
CPG = 4                      # cores per batch group
HPC = H // CPG               # 4 heads per core
JC = HPC * HD                # 512 per-core feature columns
P = 128
TB = 512                     # token block for projections
NTB = S // TB                # 4 per core (one batch)
KT = D // P                  # 16 contraction tiles
TQB = 512                    # tq block in attention
NTQB = S // TQB              # 4
NTK = S // P                 # 16 tk tiles
NBW = 256                    # output-projection free-dim block
SCALE = 1.0 / float(np.sqrt(HD))
ROPE_THETA = 10000.0

F32 = mybir.dt.float32
F16 = mybir.dt.float16
F8 = mybir.dt.float8e4

# tile classes
CLS_SKIP, CLS_ZERO, CLS_MIXED = 0, 1, 2

_prog_cache: dict[tuple, object] = {}


def _build_program(cls: np.ndarray, uidseq: tuple, n_uniq: int):
    """cls: [NTK, NTQB] int8 tile classes (shared by both batches).
    uidseq: for each mixed tile in (tqb, tk) scan order, the index of its
    mask pattern inside the resident em tensor."""
    nc = bacc.Bacc(
        "TRN2",
        target_bir_lowering=False,
        debug=False,
        enable_asserts=True,
        num_devices=NCORES,
    )

    hsT_d = nc.dram_tensor("hsT", [D, S], F16, kind="ExternalInput").ap()
    wq_d = nc.dram_tensor("wq", [P, KT * JC], F16, kind="ExternalInput").ap()
    wk_d = nc.dram_tensor("wk", [P, KT * JC], F16, kind="ExternalInput").ap()
    wv_d = nc.dram_tensor("wv", [P, KT * JC], F16, kind="ExternalInput").ap()
    wo_d = nc.dram_tensor("wo", [P, HPC * D], F16, kind="ExternalInput").ap()
    cos_d = nc.dram_tensor("cosT", [HD, S], F16, kind="ExternalInput").ap()
    sin_d = nc.dram_tensor("sinT", [HD, S], F16, kind="ExternalInput").ap()
    em_d = nc.dram_tensor("emU", [P, max(n_uniq, 1) * TQB], F16,
                          kind="ExternalInput").ap()
    out_d = nc.dram_tensor("out", [S, D], F16, kind="ExternalOutput").ap()

    hsT_v = hsT_d.rearrange("(kt p) t -> p kt t", p=P)       # [128, 16, 2048]

    # mixed-tile (uid, first-live-col) lookup in (tqb, tk) scan order
    uid_of = {}
    lo_of = {}
    ui = 0
    for tqb in range(NTQB):
        for tk in range(NTK):
            if cls[tk, tqb] == CLS_MIXED:
                uid_of[(tk, tqb)], lo_of[(tk, tqb)] = uidseq[ui]
                ui += 1
    assert ui == len(uidseq)

    FH = 2                       # heads per attention flight
    NFL = HPC // FH              # 2 flights

    with tile.TileContext(nc) as tc, ExitStack() as ctx:
        # ---------- long-lived tiles ----------
        persist = ctx.enter_context(tc.tile_pool(name="persist", bufs=1))
        qT = persist.tile([P, HPC * S], F16)      # [hd, (h, t)]
        kT = persist.tile([P, HPC * S], F16)
        vT = persist.tile([P, HPC * NTK * P], F16)  # [tok%128, (h, tk, hd)]
        ident = persist.tile([P, P], F16)
        # ones2[:, hh, :]: column 32*hh all-ones. Lands flight-head hh's
        # exp-sum on partition 32*hh of a shared [33, TQB] PSUM region
        # (engines may only address partition offsets 0/32/64/96).
        ones4 = persist.tile([P, HPC * 97], F16)
        ones4_v = ones4[:].rearrange("p (h c) -> p h c", h=HPC)


        qT_v = qT[:].rearrange("p (h t) -> p h t", h=HPC)
        kT_v = kT[:].rearrange("p (h t) -> p h t", h=HPC)
        vT_v = vT[:].rearrange("p (h m j) -> p h m j", h=HPC, m=NTK)

        aT = persist.tile([P, HPC * S], F16)      # attn out^T [hd, (h, t)]
        aT_v = aT[:].rearrange("p (h t) -> p h t", h=HPC)
        wo_s = persist.tile([P, HPC * D], F16)
        wo_sv = wo_s[:].rearrange("p (h n) -> p h n", h=HPC)
        em_s = persist.tile([P, max(n_uniq, 1) * TQB], F16)

        dbg_rbcs: list = []
        dbg_extra: list = []

        # ---------- fused phase: projections + attention per token block ----
        KQ = 4                  # kt tiles per quarter; deps are tile-granular
        NQ = KT // KQ
        sps = ctx.enter_context(
            tc.tile_pool(name="spsum", bufs=3, space="PSUM"))
        phase1 = ExitStack()
        wpool = phase1.enter_context(tc.tile_pool(name="wpool", bufs=1))
        cspool = phase1.enter_context(tc.tile_pool(name="cspool", bufs=1))
        hstp = phase1.enter_context(tc.tile_pool(name="hstp", bufs=2 * NQ))
        stg = phase1.enter_context(tc.tile_pool(name="stg", bufs=2))
        vstgp = phase1.enter_context(tc.tile_pool(name="vstg", bufs=HPC))
        pps = phase1.enter_context(
            tc.tile_pool(name="ppsum", bufs=3, space="PSUM"))
        vtp = phase1.enter_context(
            tc.tile_pool(name="vtpsum", bufs=2, space="PSUM"))
        if True:
            # weights and hs stream in kt-quarters so the PE can start after
            # ~1MB instead of waiting for whole-tile DMAs
            w_tiles = {w: [wpool.tile([P, KQ * JC], F16, name=f"w_{w}{qq}")
                           for qq in range(NQ)]
                       for w in ("v", "q", "k")}
            w_dram = {"v": wv_d, "q": wq_d, "k": wk_d}
            cos_s = cspool.tile([HD, S], F16)
            sin_s = cspool.tile([HD, S], F16)

            def load_hst_quarter(tb, qq):
                hq = hstp.tile([P, KQ * TB], F16, tag="hst")
                nc.sync.dma_start(
                    hq[:].rearrange("p (kt t) -> p kt t", t=TB),
                    hsT_v[:, qq * KQ:(qq + 1) * KQ,
                          tb * TB:(tb + 1) * TB],
                )
                return hq[:].rearrange("p (kt t) -> p kt t", t=TB)

            def load_w_quarter(w, qq):
                nc.sync.dma_start(
                    w_tiles[w][qq][:],
                    w_dram[w][:, qq * KQ * JC:(qq + 1) * KQ * JC])

            # interleaved, and the very first quarter split in halves on
            # separate DMA queues, so the first matmul is gated on ~0.5MB
            hst_q = []
            for qq in range(NQ):
                if qq == 0:
                    hq = hstp.tile([P, KQ * TB], F16, tag="hst")
                    hqv = hq[:].rearrange("p (kt t) -> p kt t", t=TB)
                    for hf in range(2):
                        nc.sync.dma_start(
                            hqv[:, 2 * hf:2 * hf + 2, :],
                            hsT_v[:, 2 * hf:2 * hf + 2, 0:TB])
                    hst_q.append(hqv)
                    for hf in range(2):
                        nc.sync.dma_start(
                            w_tiles["v"][0][:, hf * 2 * JC:(hf + 1) * 2 * JC],
                            w_dram["v"][:, hf * 2 * JC:(hf + 1) * 2 * JC])
                    continue
                hst_q.append(load_hst_quarter(0, qq))
                load_w_quarter("v", qq)
            nc.sync.dma_start(cos_s[:], cos_d)
            nc.sync.dma_start(sin_s[:], sin_d)
            for w in ("q", "k"):
                for qq in range(NQ):
                    load_w_quarter(w, qq)
            nc.scalar.dma_start(em_s[:], em_d)
            nc.scalar.dma_start(wo_s[:], wo_d)

            w_views = {
                w: [t[:].rearrange("p (kt j) -> p kt j", j=JC)
                    for t in w_tiles[w]]
                for w in ("v", "q", "k")
            }

            def proj_group(wname, j2, hst_v, dst_ps):
                for kt in range(KT):
                    nc.tensor.matmul(
                        dst_ps[:],
                        lhsT=w_views[wname][kt // KQ][
                            :, kt % KQ, j2 * P:(j2 + 1) * P],
                        rhs=hst_v[kt // KQ][:, kt % KQ, :],
                        start=(kt == 0),
                        stop=(kt == KT - 1),
                    )

            def rope_evict(j2, dst, ps, tsl):
                # out[:64] = x1*cos - x2*sin ; out[64:] = x2*cos + x1*sin
                c1 = stg.tile([P, TB], F32, tag="ropeA")
                c2 = stg.tile([P, TB], F32, tag="ropeB")
                nc.vector.tensor_mul(c1[:], ps[:], cos_s[:, tsl])
                nc.vector.tensor_mul(
                    c2[0:64, :], ps[64:128, :], sin_s[0:64, tsl])
                nc.vector.tensor_mul(
                    c2[64:128, :], ps[0:64, :], sin_s[64:128, tsl])
                nc.vector.tensor_sub(dst[0:64, :], c1[0:64, :], c2[0:64, :])
                nc.vector.tensor_add(
                    dst[64:128, :], c1[64:128, :], c2[64:128, :])

            def attention_block(tqb, sps, sups, ptp, ptmp, smp, rbcp,
                                emit_oproj_groups):
                tq0 = tqb * TQB
                live = [tk for tk in range(NTK) if cls[tk, tqb] != CLS_SKIP]
                L = len(live)
                los = [lo_of.get((tk, tqb), 0) for tk in live]

                # pair consecutive full (ZERO) tiles for fp8 DoubleRow
                # denominator sums; mask tiles keep the fp16 ones-matmul
                pair_second = {}
                zrun = []  # fp8-DR denominator rejected: softmax
                # concentration lets a dominant key's e4m3 error (~6%)
                # reach the output unaveraged
                zi = 0
                while zi + 1 < len(zrun):
                    if zrun[zi + 1] == zrun[zi] + 1:
                        pair_second[zrun[zi + 1]] = zrun[zi]
                        zi += 2
                    else:
                        zi += 1
                sums_events = [i for i in range(L)
                               if i not in pair_second.values()]

                def av_sums(i, hh, h, pt, sums, fl):
                    # one accumulation window per PSUM bank: tile 0 zeroes
                    # the full range (los[0] == 0 guaranteed), the last tile
                    # closes it; middle tiles write their live subrange only
                    first = (i == 0)
                    last = (i == L - 1)
                    lo = 0 if first else los[i]
                    nc.tensor.matmul(
                        o_ps[hh][:, lo:],
                        lhsT=vT_v[:, h, live[i], :],
                        rhs=pt[:, lo:],
                        start=first, stop=last,
                    )
                    if i in pair_second.values():
                        return
                    s_first = (i == sums_events[0])
                    s_last = (i == sums_events[-1])
                    nc.tensor.matmul(
                        sums[:, lo:],
                        lhsT=ones4_v[:, fl * FH + hh, :],
                        rhs=pt[:, lo:],
                        start=(s_first and first and hh == 0),
                        stop=(s_last and last and hh == FH - 1),
                    )

                for fl in range(NFL):
                    heads = range(fl * FH, (fl + 1) * FH)
                    sums = sups.tile([97, TQB], F32, tag="sums")
                    pts = [[None] * FH for _ in range(2)]
                    for i, tk in enumerate(live):
                        lo = los[i]
                        for hh, h in enumerate(heads):
                            st = sps.tile([P, TQB], F32, tag="st")
                            nc.tensor.matmul(
                                st[:, lo:],
                                lhsT=kT_v[:, h, tk * P:(tk + 1) * P],
                                rhs=qT_v[:, h, tq0 + lo:tq0 + TQB],
                                start=True, stop=True,
                            )
                            pt = ptp.tile([P, TQB], F16, tag="pt")
                            nc.scalar.activation(
                                pt[:, lo:], st[:, lo:],
                                mybir.ActivationFunctionType.Exp,
                                scale=SCALE,
                            )
                            if cls[tk, tqb] == CLS_MIXED:
                                # out-of-place: PE never sees pre-mask pt
                                u = uid_of[(tk, tqb)]
                                ptm = ptmp.tile([P, TQB], F16, tag="ptm")
                                nc.vector.tensor_mul(
                                    ptm[:, lo:], pt[:, lo:],
                                    em_s[:, u * TQB + lo:(u + 1) * TQB])
                                pt = ptm
                            pts[i % 2][hh] = pt
                            if i > 0:
                                av_sums(i - 1, hh, h,
                                        pts[(i - 1) % 2][hh], sums, fl)
                    for hh, h in enumerate(heads):
                        av_sums(L - 1, hh, h, pts[(L - 1) % 2][hh], sums, fl)
                    dbg_sums = None
                    if os.environ.get("KERNEL_DEBUG") and tqb == NTQB - 1:
                        dbg_sums = persist.tile([97, TQB], F32)
                        nc.scalar.copy(dbg_sums[:], sums[:])
                        dbg_extra.append((f"dbg_sums{fl}", dbg_sums))
                    for hh, h in enumerate(heads):
                        # reciprocal_approx_fast mis-reads PSUM partition
                        # offsets != 0 on HW: stage the row to partition 0
                        sr = smp.tile([1, TQB], F32, tag="sr")
                        r0 = 32 * (fl * FH + hh)
                        nc.scalar.copy(sr[:], sums[r0:r0 + 1, :])
                        rc = smp.tile([1, TQB], F32, tag="rc")
                        nc.vector.reciprocal_approx_fast(rc[:], sr[:])
                        rbc = rbcp.tile([P, TQB], F32, tag="rbc")
                        nc.gpsimd.partition_broadcast(rbc[:], rc[:])
                        if dbg_sums is not None:
                            dbg_rbcs.append(rbc)
                        nc.vector.tensor_mul(
                            aT_v[:, h, tq0:tq0 + TQB], o_ps[hh][:], rbc[:])
                    emit_oproj_groups(8)

            for tb in range(NTB):
                if tb > 0:
                    hst_q = [load_hst_quarter(tb, qq) for qq in range(NQ)]
                tsl = slice(tb * TB, (tb + 1) * TB)
                # v first: its fp16 staging copies run while q projects; the
                # v chunks transpose on the DMA xbar (not the PE)
                vss = []
                for j2 in range(HPC):
                    ps = pps.tile([P, TB], F32, tag="pp")
                    proj_group("v", j2, hst_q, ps)
                    vs = vstgp.tile([P, TB], F16, tag="vstage")
                    nc.scalar.copy(vs[:], ps[:])
                    vss.append(vs)
                    if tb == 0 and j2 == 0:
                        # deferred off the startup critical path
                        make_identity(nc, ident[:])
                        nc.gpsimd.memset(ones4[:], 0.0)
                        for h4 in range(HPC):
                            nc.gpsimd.memset(
                                ones4_v[:, h4, 32 * h4:32 * h4 + 1], 1.0)
                for j2 in range(HPC):
                    ps = pps.tile([P, TB], F32, tag="pp")
                    proj_group("q", j2, hst_q, ps)
                    rope_evict(j2, qT_v[:, j2, tsl], ps, tsl)
                for j2 in range(HPC):
                    for k in range(TB // P):
                        vps = vtp.tile([P, P], F16, tag="vt")
                        nc.tensor.matmul(
                            vps[:],
                            lhsT=vss[j2][:, k * P:(k + 1) * P],
                            rhs=ident[:],
                            is_transpose=True,
                        )
                        nc.scalar.copy(
                            vT_v[:, j2, tb * (TB // P) + k, :], vps[:])
                for j2 in range(HPC):
                    ps = pps.tile([P, TB], F32, tag="pp")
                    proj_group("k", j2, hst_q, ps)
                    rope_evict(j2, kT_v[:, j2, tsl], ps, tsl)
            phase1.close()

        # ---------- phase 2: attention with interleaved output projection ----
        with tc.tile_pool(name="ptp", bufs=8) as ptp, \
             tc.tile_pool(name="ptmp", bufs=4) as ptmp, \
             tc.tile_pool(name="smp", bufs=2) as smp, \
             tc.tile_pool(name="rbcp", bufs=4) as rbcp, \
             tc.tile_pool(name="ostg", bufs=3) as ostgp, \
             tc.tile_pool(name="opsum", bufs=2, space="PSUM") as ops, \
             tc.tile_pool(name="avpsum", bufs=1, space="PSUM") as avp, \
             tc.tile_pool(name="supsum", bufs=1, space="PSUM") as sups:
            o_ps = [avp.tile([P, TQB], F32, tag=f"av{hh}", name=f"o_ps{hh}")
                    for hh in range(FH)]

            pending: list = []
            ng = [0]

            def emit_oproj_groups(n):
                for _ in range(min(n, len(pending))):
                    tb32, nb = pending.pop(0)
                    pso = ops.tile([P, TQB], F32, tag="op")
                    for j2 in range(HPC):
                        nc.tensor.matmul(
                            pso[:],
                            lhsT=aT_v[:, j2, tb32 * P:(tb32 + 1) * P],
                            rhs=wo_sv[:, j2, nb * TQB:(nb + 1) * TQB],
                            start=(j2 == 0), stop=(j2 == HPC - 1),
                        )
                    og = ostgp.tile([P, TQB], F16, tag="og")
                    # alternate eviction engine to balance load
                    if ng[0] % 2 == 0:
                        nc.vector.tensor_copy(og[:], pso[:])
                    else:
                        nc.scalar.copy(og[:], pso[:])
                    (nc.sync if ng[0] % 2 == 0 else nc.scalar).dma_start(
                        out_d[tb32 * P:(tb32 + 1) * P,
                              nb * TQB:(nb + 1) * TQB],
                        og[:],
                    )
                    ng[0] += 1

            for tqb in range(NTQB):
                attention_block(tqb, sps, sups, ptp, ptmp, smp, rbcp,
                                emit_oproj_groups)
                pending += [(tb32, nb)
                            for tb32 in range(tqb * (TQB // P),
                                              (tqb + 1) * (TQB // P))
                            for nb in range(D // TQB)]
            emit_oproj_groups(len(pending))

            if os.environ.get("KERNEL_DEBUG"):
                for nm, t in (("dbg_qT", qT), ("dbg_kT", kT),
                              ("dbg_vT", vT), ("dbg_aT", aT),
                              ("dbg_em", em_s)):
                    dd = nc.dram_tensor(
                        nm, list(t[:].shape), F16, kind="ExternalOutput").ap()
                    nc.sync.dma_start(dd, t[:])
                for hh, rb in enumerate(dbg_rbcs):
                    dd = nc.dram_tensor(
                        f"dbg_rbc{hh}", [P, TQB], F32,
                        kind="ExternalOutput").ap()
                    nc.sync.dma_start(dd, rb[:])
                for nm, t in dbg_extra:
                    dd = nc.dram_tensor(
                        nm, list(t[:].shape), F32, kind="ExternalOutput").ap()
                    nc.sync.dma_start(dd, t[:])

    nc.compile()
    return nc


def _host_prep(hidden_states, attention_mask, position_ids):
    hs2 = np.asarray(hidden_states, dtype=np.float32).reshape(B * S, D)
    hsT = np.ascontiguousarray(hs2.T).astype(ml_dtypes.float16 if False
                                             else np.float16)  # [D, B*S]

    # rope tables gathered by position_ids, feature-major
    inv_freq = 1.0 / (ROPE_THETA ** (np.arange(0, HD, 2, dtype=np.float32) / HD))
    pos = np.asarray(position_ids).astype(np.int64)
    maxpos = int(pos.max()) + 1
    t_ar = np.arange(maxpos, dtype=np.float32)
    freqs = np.outer(t_ar, inv_freq)
    emb = np.concatenate([freqs, freqs], axis=-1)           # [maxpos, 128]
    cos_tab = np.cos(emb).astype(np.float32)
    sin_tab = np.sin(emb).astype(np.float32)
    cosT = [np.ascontiguousarray(cos_tab[pos[b]].T).astype(np.float16)
            for b in range(B)]                               # [HD, S] each
    sinT = [np.ascontiguousarray(sin_tab[pos[b]].T).astype(np.float16)
            for b in range(B)]

    # shifted-exp mask, transposed per batch, tile classification + dedup
    m = np.asarray(attention_mask, dtype=np.float32)[:, 0]  # [B, tq, tk]
    rowmax = m.max(axis=-1, keepdims=True)
    em = np.exp(m - rowmax)                                 # [B, tq, tk] in [0,1]
    emT = np.ascontiguousarray(em.transpose(0, 2, 1))       # [B, tk, tq]
    emr = emT.reshape(B, NTK, P, NTQB, TQB)
    tmax = emr.max(axis=(2, 4))                             # [B, NTK, NTQB]
    tmin = emr.min(axis=(2, 4))
    cls_b = np.full((B, NTK, NTQB), CLS_MIXED, dtype=np.int8)
    cls_b[tmax == 0.0] = CLS_SKIP
    cls_b[(tmin == 1.0) & (tmax == 1.0)] = CLS_ZERO
    # merge across batches: process if any batch needs it; mixed if classes
    # differ or any is mixed (em data is per-batch anyway)
    cls = np.maximum(cls_b[0], cls_b[1])
    cls[cls_b[0] != cls_b[1]] = CLS_MIXED
    # guard: a fully-skipped tq column would leave PSUM unwritten
    for tqb in range(NTQB):
        if (cls[:, tqb] == CLS_SKIP).all():
            cls[0, tqb] = CLS_MIXED

    # per-batch unique mixed-tile patterns + first-live-column, scan order
    uidseqs, uniqs, loseqs = [], [], []
    for b in range(B):
        seen = {}
        seq = []
        los = []
        tiles = []
        for tqb in range(NTQB):
            for tk in range(NTK):
                if cls[tk, tqb] != CLS_MIXED:
                    continue
                blk = np.ascontiguousarray(
                    emT[b, tk * P:(tk + 1) * P,
                        tqb * TQB:(tqb + 1) * TQB]).astype(np.float16)
                key = blk.tobytes()
                if key not in seen:
                    seen[key] = len(tiles)
                    tiles.append(blk)
                seq.append(seen[key])
                nzc = blk.astype(bool).any(axis=0)
                los.append(int(np.argmax(nzc)) if nzc.any() else 0)
        uidseqs.append(tuple(seq))
        loseqs.append(los)
        uniqs.append(tiles)
    if uidseqs[0] != uidseqs[1]:
        # fall back to no dedup: sequential uids shared by construction
        nm = len(uidseqs[0])
        seqs = tuple(range(nm))
        uidseqs = [seqs, seqs]
        uniqs = [
            [np.ascontiguousarray(
                emT[b, tk * P:(tk + 1) * P, tqb * TQB:(tqb + 1) * TQB]
             ).astype(np.float16)
             for tqb in range(NTQB) for tk in range(NTK)
             if cls[tk, tqb] == CLS_MIXED]
            for b in range(B)
        ]
    # live-column offsets must agree across batches, else no restriction
    los = [a if a == bb else 0 for a, bb in zip(loseqs[0], loseqs[1])]
    # per tq block: offsets must start at 0 (zero-class tiles are implicit 0)
    # and be non-decreasing in tk scan order, else disable for that block
    ui = 0
    for tqb in range(NTQB):
        idxs = []
        full = []
        for tk in range(NTK):
            if cls[tk, tqb] == CLS_MIXED:
                full.append(los[ui])
                idxs.append(ui)
                ui += 1
            elif cls[tk, tqb] == CLS_ZERO:
                full.append(0)
        ok = all(full[i] <= full[i + 1] for i in range(len(full) - 1))
        if full and full[0] != 0:
            ok = False
        if not ok:
            for j in idxs:
                los[j] = 0
    uidseq = tuple(zip(uidseqs[0], los))
    n_uniq = max(len(uniqs[0]), len(uniqs[1]), 1)
    em_u = []
    for b in range(B):
        buf = np.zeros((P, n_uniq * TQB), dtype=np.float16)
        for u, blk in enumerate(uniqs[b]):
            buf[:, u * TQB:(u + 1) * TQB] = blk
        em_u.append(buf)
    return hsT, cosT, sinT, cls, uidseq, n_uniq, em_u


def _sb_w(w):  # [D, JC] fp32 -> SBUF layout [128, KT*JC] fp16
    return np.ascontiguousarray(
        w.reshape(KT, P, JC).transpose(1, 0, 2).reshape(P, KT * JC)
    ).astype(np.float16)


def kernel(hidden_states, attention_mask, position_ids, Wq, Wk, Wv, Wo):
    hsT, cosT, sinT, cls, uidseq, n_uniq, em_u = _host_prep(
        hidden_states, attention_mask, position_ids)

    key = (cls.tobytes(), uidseq, n_uniq)
    if key not in _prog_cache:
        _prog_cache[key] = _build_program(cls, uidseq, n_uniq)
    nc = _prog_cache[key]

    Wq = np.asarray(Wq, dtype=np.float32)
    Wk = np.asarray(Wk, dtype=np.float32)
    Wv = np.asarray(Wv, dtype=np.float32)
    Wo = np.asarray(Wo, dtype=np.float32)

    in_maps = []
    for c in range(NCORES):
        b, g = c // CPG, c % CPG
        jsl = slice(g * JC, (g + 1) * JC)
        m = {
            "hsT": np.ascontiguousarray(hsT[:, b * S:(b + 1) * S]),
            "wq": _sb_w(Wq[:, jsl]),
            "wk": _sb_w(Wk[:, jsl]),
            "wv": _sb_w(Wv[:, jsl]),
            "wo": np.ascontiguousarray(
                Wo[jsl, :].reshape(HPC, P, D).transpose(1, 0, 2)
                .reshape(P, HPC * D)).astype(np.float16),
            "cosT": cosT[b],
            "sinT": sinT[b],
            "emU": em_u[b],
        }
        in_maps.append(m)

    if os.environ.get("KERNEL_SIM"):
        from concourse.bass_interp import CoreSim
        outs = []
        for c in range(int(os.environ.get("KERNEL_SIM_CORES", "1"))):
            sim = CoreSim(nc, require_finite=False, require_nnan=True)
            for k, v in in_maps[c].items():
                sim.tensor(k)[:] = v
            sim.simulate(check_with_hw=False)
            outs.append(np.array(sim.tensor("out")).astype(np.float32))
        kernel.last_sim_partials = outs
        total = np.zeros((B, S, D), dtype=np.float32)
        for c, o in enumerate(outs):
            total[c // CPG] += o
        return total

    from concourse.bass_utils import run_bass_kernel_spmd
    trace = bool(os.environ.get("KERNEL_TRACE"))
    res = run_bass_kernel_spmd(
        nc, in_maps, core_ids=list(range(NCORES)), trace=trace)
    if trace and res.exec_time_ns is not None:
        print(f"HW exec time: {res.exec_time_ns} ns")
        kernel.last_exec_time_ns = res.exec_time_ns
        kernel.last_trace = res.instructions_and_trace
    total = np.zeros((B, S, D), dtype=np.float32)
    for c, r in enumerate(res.results):
        total[c // CPG] += np.asarray(r["out"], dtype=np.float32)
    return total
